# revision 2
# baseline (speedup 1.0000x reference)
"""Trainium2 Bass kernel for the Competitive Progressive Temporal Module.

Reference computation (per sample):
  f1 = relu(conv_t(x,  w1) + b1)        # temporal conv, kernel 3, SAME
  f2 = relu(conv_t(f1, w2) + b2)
  f3 = relu(conv_t(f2, w3) + b3)
  s  = mean_{t,h,w}((f1+f2+f3)/3)                         # (C,)
  h  = relu(bn(s @ fc_w))                                 # (D,)
  att= softmax_b(h @ fcs_w[b] + fcs_b[b])                 # (3, C)
  out[t,c,hw] = sum_b att[b,c] * f_b[c,t,hw]

Distribution: data-parallel over N=8 samples across 8 NeuronCores (params
replicated, no cross-core communication).

Per-core layout: SBUF tiles are [(parity,c)=128 partitions, frame-pair, s].
Partition p*64+c holds channel c of frames with t%2==p.  For one column
block q (frames 2q, 2q+1), a single K=128, M=128 matmul computes the two
"main" taps of BOTH output frames 2q (PSUM partitions 0:64) and 2q+1
(64:128); the remaining tap of each output comes from a neighboring block
via a K=64 "tail" matmul on one parity half.  SAME padding falls out by
skipping the out-of-range tails at t=0 / t=T-1.

conv1 consumes x directly as float32r (full-rate on PE for N>=256), so x is
never cast; conv2/3 run in fp16 on the archived fp16 activations.  f1/f2/f3
are archived in SBUF as fp16; x is read from HBM once and the output
written once (memory-roofline regime).

Pass B (out = sum_b att_b * f_b) runs on the otherwise-idle PE as three
accumulated matmuls per block with diagonal lhsT = diag(att_b), built by
scaling a constant identity by the attention vector; drains alternate
ACT/DVE.  The squeeze/attention head runs in fp32.
"""

import numpy as np

import concourse.bass as bass
import concourse.bacc as bacc
import concourse.tile as tile
from concourse import mybir
from concourse.bass_utils import run_bass_kernel_spmd

# Problem constants (hardcoded per harness contract).
B = 3          # branches
C = 64         # channels
D = 32         # bottleneck dim
T = 16         # frames
HW = 56 * 56   # spatial
SC = 392       # spatial columns per tile
NT = HW // SC  # 8 spatial tiles
Q = T // 2     # frame pairs per tile (= 8 column blocks)
NCORES = 8
BN_EPS = 1e-3

F32 = mybir.dt.float32
F32R = mybir.dt.float32r
F16 = mybir.dt.float16
AX = mybir.AxisListType
OP = mybir.AluOpType
AF = mybir.ActivationFunctionType


def _emit_conv_kind(nc, ps, in_even, in_odd, in_full, lhsT_mains, lhsT_tails,
                    quarter, kind):
    """Emit one weight-kind (0=mains, 1=TE, 2=TO) of one conv for one psum
    quarter (2 frame-pair blocks)."""
    q0 = 2 * quarter
    for j in range(2):
        q = q0 + j
        if kind == 0:
            nc.tensor.matmul(ps[0:128, j, 0:SC], lhsT=lhsT_mains,
                             rhs=in_full[:, q, :], start=True, stop=False,
                             skip_group_check=True)
        elif kind == 1 and q >= 1:
            nc.tensor.matmul(ps[0:64, j, 0:SC], lhsT=lhsT_tails[64:128, :],
                             rhs=in_odd[:, q - 1, :], start=False, stop=True,
                             skip_group_check=True)
        elif kind == 2 and q <= 6:
            nc.tensor.matmul(ps[64:128, j, 0:SC], lhsT=lhsT_tails[0:64, :],
                             rhs=in_even[:, q + 1, :], start=False, stop=True,
                             skip_group_check=True)


def _build_module(reps=1):
    nc = bacc.Bacc("TRN2", target_bir_lowering=False, debug=False,
                   num_devices=NCORES)

    x_d = nc.dram_tensor("x", [C, T, HW], F32R, kind="ExternalInput")
    out_d = nc.dram_tensor("out", [T, C, HW], F32, kind="ExternalOutput")
    w1_d = nc.dram_tensor("wconv1", [128, 192], F32R, kind="ExternalInput")
    w_d = nc.dram_tensor("wconv23", [128, 384], F16, kind="ExternalInput")
    eye_d = nc.dram_tensor("eye128", [128, 128], F16, kind="ExternalInput")
    bias_d = nc.dram_tensor("bias128", [128, B], F32, kind="ExternalInput")
    nbias_d = nc.dram_tensor("nbias128", [128, B], F32, kind="ExternalInput")
    fcw_d = nc.dram_tensor("fcw128", [128, D], F32, kind="ExternalInput")
    bn_d = nc.dram_tensor("bnsb", [D, 2], F32, kind="ExternalInput")
    fcs_d = nc.dram_tensor("fcs_lhsT", [D, B, 128], F32, kind="ExternalInput")
    fcsb_d = nc.dram_tensor("fcsb128", [128, B], F32, kind="ExternalInput")

    # [p c q u s] views of the HBM tensors; DMA'd per parity half so the
    # partition dim (c) has a single stride.
    x_v = x_d.ap().rearrange("c (q p) (u s) -> p c q u s", p=2, s=SC)
    out_v = out_d.ap().rearrange("(q p) c (u s) -> p c q u s", p=2, s=SC)

    with tile.TileContext(nc) as tc:
        with (
            tc.tile_pool(name="consts", bufs=1) as consts,
            tc.tile_pool(name="arch", bufs=1) as archp,
            tc.tile_pool(name="xin", bufs=2) as xin,
            tc.tile_pool(name="outp", bufs=2) as outp,
            tc.tile_pool(name="small", bufs=1) as small,
            tc.tile_pool(name="psum", bufs=4, space="PSUM") as psump,
        ):
            w1_sb = consts.tile([128, 192], F32R, tag="w1", name="w1")
            w_sb = consts.tile([128, 384], F16, tag="w", name="w")
            eye_sb = consts.tile([128, 128], F16, tag="eye", name="eye")
            bias_sb = consts.tile([128, B], F32, tag="bias", name="bias")
            nbias_sb = consts.tile([128, B], F32, tag="nbias", name="nbias")
            fcw_sb = consts.tile([128, D], F32, tag="fcw", name="fcw")
            bn_sb = consts.tile([D, 2], F32, tag="bn", name="bn")
            fcs_sb = consts.tile([D, B, 128], F32, tag="fcs", name="fcs")
            fcsb_sb = consts.tile([128, B], F32, tag="fcsb", name="fcsb")
            acc = consts.tile([128, 96], F32, tag="acc", name="acc")
            nc.sync.dma_start(out=w1_sb, in_=w1_d.ap())
            nc.sync.dma_start(out=w_sb, in_=w_d.ap())
            nc.sync.dma_start(out=eye_sb, in_=eye_d.ap())
            nc.sync.dma_start(out=bias_sb, in_=bias_d.ap())
            nc.sync.dma_start(out=nbias_sb, in_=nbias_d.ap())
            nc.sync.dma_start(out=fcw_sb, in_=fcw_d.ap())
            nc.sync.dma_start(out=bn_sb, in_=bn_d.ap())
            nc.sync.dma_start(out=fcs_sb, in_=fcs_d.ap())
            nc.sync.dma_start(out=fcsb_sb, in_=fcsb_d.ap())

            # Persistent fp16 archives of f1/f2/f3 (whole sample).
            arch = [archp.tile([128, NT, Q, SC], F16, tag=f"arch{i}",
                               name=f"arch{i}") for i in range(B)]

            # Per conv: mains [128, 0:128], tails [128, 128:192].
            conv_w = [(w1_sb[:, 0:128], w1_sb[:, 128:192]),
                      (w_sb[:, 0:128], w_sb[:, 128:192]),
                      (w_sb[:, 192:320], w_sb[:, 320:384])]

            for _rep in range(reps):
                # ---------------- Pass A: convs + channel sums ----------------
                # Skewed software pipeline: wave w emits conv ci of tile
                # w-ci, so conv1(u+1) interleaves with conv2(u)/conv3(u-1)
                # and the PE always has ready matmuls while drains complete.
                slot = 0

                xts = {}

                def conv_inputs(ci, u):
                    a = xts[u] if ci == 0 else arch[ci - 1][:, u]
                    return a[0:64], a[64:128], a

                for w in range(NT + B - 1):
                    if w < NT:
                        x_t = xin.tile([128, Q, SC], F32R, tag="x", name="x")
                        nc.sync.dma_start(out=x_t[0:64], in_=x_v[0, :, :, w, :])
                        nc.sync.dma_start(out=x_t[64:128],
                                          in_=x_v[1, :, :, w, :])
                        xts[w] = x_t
                    for ci in range(B):
                        u = w - ci
                        if not (0 <= u < NT):
                            continue
                        mains, tails = conv_w[ci]
                        in_even, in_odd, in_full = conv_inputs(ci, u)
                        for g in range(4):
                            ps = psump.tile([128, 2, 512], F32, tag="psum",
                                            name="psum")
                            for kind in range(3):
                                _emit_conv_kind(nc, ps, in_even, in_odd,
                                                in_full, mains, tails, g,
                                                kind)
                            dst = arch[ci][:, u, 2 * g:2 * g + 2, :]
                            # Drain+ReLU+channel-sum; route one quarter per
                            # conv-tile to the DVE as (x max -b) add b ==
                            # relu(x+b), the rest to ACT (bias free via the
                            # activation affine).
                            if g == 1:
                                nc.vector.tensor_scalar(
                                    out=dst, in0=ps[:, :, 0:SC],
                                    scalar1=nbias_sb[:, ci:ci + 1],
                                    scalar2=bias_sb[:, ci:ci + 1],
                                    op0=OP.max, op1=OP.add,
                                    accum_out=acc[:, slot:slot + 1])
                            else:
                                nc.scalar.activation(
                                    out=dst, in_=ps[:, :, 0:SC],
                                    func=AF.Relu,
                                    bias=bias_sb[:, ci:ci + 1], scale=1.0,
                                    accum_out=acc[:, slot:slot + 1])
                            slot += 1
                    if w >= B - 1:
                        xts.pop(w - (B - 1), None)

                # ---------------- Head: s -> h -> att ----------------
                red = small.tile([128, 1], F32, tag="red", name="red")
                nc.vector.tensor_reduce(out=red, in_=acc[:, 0:slot], axis=AX.X,
                                        op=OP.add)
                ps_h = psump.tile([128, 2, 512], F32, tag="psum", name="psum")
                # h = fc_w128^T @ red  (scale 1/(3*T*HW) folded into fc_w128)
                nc.tensor.matmul(ps_h[0:32, 0, 0:1], lhsT=fcw_sb, rhs=red,
                                 start=True, stop=True)
                h_sb = small.tile([D, 1], F32, tag="h", name="h")
                nc.scalar.activation(out=h_sb, in_=ps_h[0:32, 0, 0:1],
                                     func=AF.Relu,
                                     bias=bn_sb[:, 1:2], scale=bn_sb[:, 0:1])
                for b in range(B):
                    nc.tensor.matmul(ps_h[:, 1, b:b + 1], lhsT=fcs_sb[:, b, :],
                                     rhs=h_sb, start=True, stop=True)
                logits = small.tile([128, B], F32, tag="logits", name="logits")
                nc.vector.tensor_tensor(out=logits, in0=ps_h[:, 1, 0:B],
                                        in1=fcsb_sb, op=OP.add)
                mx = small.tile([128, 1], F32, tag="mx", name="mx")
                nc.vector.tensor_reduce(out=mx, in_=logits, axis=AX.X, op=OP.max)
                negmx = small.tile([128, 1], F32, tag="negmx", name="negmx")
                nc.vector.tensor_scalar(out=negmx, in0=mx, scalar1=-1.0,
                                        scalar2=None, op0=OP.mult)
                e = small.tile([128, B], F32, tag="e", name="e")
                nc.scalar.activation(out=e, in_=logits, func=AF.Exp, bias=negmx,
                                     scale=1.0)
                ssum = small.tile([128, 1], F32, tag="ssum", name="ssum")
                nc.vector.tensor_reduce(out=ssum, in_=e, axis=AX.X, op=OP.add)
                rcp = small.tile([128, 1], F32, tag="rcp", name="rcp")
                nc.vector.reciprocal(out=rcp, in_=ssum)
                att = small.tile([128, B], F32, tag="att", name="att")
                nc.vector.tensor_scalar(out=att, in0=e, scalar1=rcp, scalar2=None,
                                        op0=OP.mult)
                # Diagonal attention matrices for the pass-B matmuls.
                diag = small.tile([128, B, 128], F16, tag="diag", name="diag")
                for b in range(B):
                    nc.vector.tensor_scalar(out=diag[:, b, :], in0=eye_sb,
                                            scalar1=att[:, b:b + 1],
                                            scalar2=None, op0=OP.mult)

                # ------- Pass B: out = sum_b att_b * f_b (on the PE) -------
                for u in range(NT):
                    ot = outp.tile([128, Q, SC], F32, tag="out", name="out")
                    for g in range(4):
                        ps = psump.tile([128, 2, 512], F32, tag="psum",
                                        name="psum")
                        for j in range(2):
                            q = 2 * g + j
                            for b in range(B):
                                nc.tensor.matmul(
                                    ps[:, j, 0:SC], lhsT=diag[:, b, :],
                                    rhs=arch[b][:, u, q, :],
                                    start=(b == 0), stop=(b == B - 1),
                                    skip_group_check=True)
                        dst = ot[:, 2 * g:2 * g + 2, :]
                        if g % 2 == 0:
                            nc.scalar.activation(out=dst, in_=ps[:, :, 0:SC],
                                                 func=AF.Copy, scale=1.0)
                        else:
                            nc.vector.tensor_copy(out=dst, in_=ps[:, :, 0:SC])
                    nc.sync.dma_start(out=out_v[0, :, :, u, :], in_=ot[0:64])
                    nc.sync.dma_start(out=out_v[1, :, :, u, :], in_=ot[64:128])

    nc.compile()
    return nc


_NC_CACHE = []


_NC_CACHE_R = {}


def _get_module(reps=1):
    if reps == 1:
        if not _NC_CACHE:
            _NC_CACHE.append(_build_module())
        return _NC_CACHE[0]
    if reps not in _NC_CACHE_R:
        _NC_CACHE_R[reps] = _build_module(reps)
    return _NC_CACHE_R[reps]


def _host_params(conv_w, conv_b, fc_w, bn_gamma, bn_beta, bn_mean, bn_var,
                 fcs_w, fcs_b):
    conv_w = np.asarray(conv_w, np.float32)
    conv_b = np.asarray(conv_b, np.float32)
    fc_w = np.asarray(fc_w, np.float32)
    fcs_w = np.asarray(fcs_w, np.float32)
    fcs_b = np.asarray(fcs_b, np.float32)

    def pack(i):
        w0 = conv_w[i, :, :, 0, 0, 0].T.copy()  # [ci, co]
        w1 = conv_w[i, :, :, 1, 0, 0].T.copy()
        w2 = conv_w[i, :, :, 2, 0, 0].T.copy()
        om = np.concatenate([w1, w2], axis=0)        # even outputs main
        em = np.concatenate([w0, w1], axis=0)        # odd outputs main
        mains = np.concatenate([om, em], axis=1)     # [128, 128]
        tails = np.concatenate([w2, w0], axis=0)     # TO rows 0:64, TE 64:128
        return np.concatenate([mains, tails], axis=1)  # [128, 192]

    w1_h = pack(0).astype(np.float32)
    w23_h = np.concatenate([pack(1), pack(2)], axis=1).astype(np.float16)
    eye_h = np.eye(128, dtype=np.float16)
    bias_h = np.stack([np.concatenate([conv_b[i], conv_b[i]])
                       for i in range(B)], axis=1).astype(np.float32)
    fcw_h = (np.concatenate([fc_w, fc_w], axis=0)
             / np.float32(B * T * HW)).astype(np.float32)
    bn_scale = (np.asarray(bn_gamma, np.float32)
                / np.sqrt(np.asarray(bn_var, np.float32) + BN_EPS))
    bn_bias = (np.asarray(bn_beta, np.float32)
               - np.asarray(bn_mean, np.float32) * bn_scale)
    bn_h = np.stack([bn_scale, bn_bias], axis=1).astype(np.float32)
    fcs_h = np.zeros((D, B, 128), np.float32)
    for b in range(B):
        fcs_h[:, b, 0:64] = fcs_w[b]
        fcs_h[:, b, 64:128] = fcs_w[b]
    fcsb_h = np.stack([np.concatenate([fcs_b[b], fcs_b[b]])
                       for b in range(B)], axis=1).astype(np.float32)
    return dict(wconv1=w1_h, wconv23=w23_h, eye128=eye_h, bias128=bias_h,
                nbias128=-bias_h, fcw128=fcw_h, bnsb=bn_h, fcs_lhsT=fcs_h,
                fcsb128=fcsb_h)


def make_in_maps(x, params):
    x = np.ascontiguousarray(np.asarray(x, np.float32).reshape(NCORES, C, T, HW))
    return [dict(params, x=x[n]) for n in range(NCORES)]


def gather_out(results):
    return np.concatenate(
        [r["out"].reshape(T, C, 56, 56) for r in results], axis=0)


def kernel(x, conv_w, conv_b, fc_w, bn_gamma, bn_beta, bn_mean, bn_var,
           fcs_w, fcs_b):
    nc = _get_module()
    params = _host_params(conv_w, conv_b, fc_w, bn_gamma, bn_beta, bn_mean,
                          bn_var, fcs_w, fcs_b)
    res = run_bass_kernel_spmd(nc, make_in_maps(x, params),
                               core_ids=list(range(NCORES)))
    return gather_out(res.results)
